# revision 3
# baseline (speedup 1.0000x reference)
"""ChannelAttentionModule kernel for TRN2 (Bass/Tile), 8-core SPMD.

Computes sigmoid(mean_{hw}(x) @ W.T + b) for x:[16,128,256,256].
Sharding: data-parallel over batch, 2 images per core. Each core streams
its 64 MiB shard once from HBM, reduces over H*W on the vector engine,
contracts channels with W on the tensor engine, applies sigmoid on the
scalar engine. Output [16,1,1,1] gathered on host by concatenation.
"""

import numpy as np

_B, _C, _HW = 16, 128, 65536  # batch, channels, H*W
_NCORES = 8
_BPC = _B // _NCORES  # batches per core = 2
_NCH = 8  # chunks per batch
_F = _HW // _NCH  # free-dim elements per chunk = 8192

_cached_nc = None


def _build_nc():
    import concourse.bacc as bacc
    import concourse.tile as tile
    from concourse import mybir

    f32 = mybir.dt.float32
    nc = bacc.Bacc(
        "TRN2",
        target_bir_lowering=False,
        debug=False,
        num_devices=_NCORES,
    )

    x = nc.dram_tensor("x", [_BPC, _C, _HW], f32, kind="ExternalInput")
    w = nc.dram_tensor("w", [_C, 1], f32, kind="ExternalInput")
    bvec = nc.dram_tensor("bias", [1, 1], f32, kind="ExternalInput")
    out = nc.dram_tensor("out", [1, _BPC], f32, kind="ExternalOutput")

    with tile.TileContext(nc) as tc:
        with (
            tc.tile_pool(name="big", bufs=4) as big,
            tc.tile_pool(name="small", bufs=1) as small,
            tc.tile_pool(name="psum", bufs=1, space="PSUM") as psum,
        ):
            w_sb = small.tile([_C, 1], f32)
            nc.sync.dma_start(out=w_sb[:], in_=w[:])
            b_sb = small.tile([1, 1], f32)
            nc.sync.dma_start(out=b_sb[:], in_=bvec[:])

            partials = small.tile([_C, _BPC * _NCH], f32)
            for bi in range(_BPC):
                for j in range(_NCH):
                    t = big.tile([_C, _F], f32, tag="xtile")
                    nc.sync.dma_start(out=t[:], in_=x[bi, :, j * _F : (j + 1) * _F])
                    k = bi * _NCH + j
                    nc.vector.reduce_sum(
                        out=partials[:, k : k + 1],
                        in_=t[:],
                        axis=mybir.AxisListType.X,
                    )

            # Contract channels: [1, BPC*NCH] = w[C,1].T @ partials[C, BPC*NCH]
            ps = psum.tile([1, _BPC * _NCH], f32)
            nc.tensor.matmul(ps[:], w_sb[:], partials[:])

            # Sum each batch's chunk partials: [1, BPC]
            att = small.tile([1, _BPC], f32)
            nc.vector.reduce_sum(
                out=att[:],
                in_=ps[:].rearrange("p (b j) -> p b j", j=_NCH),
                axis=mybir.AxisListType.X,
            )

            # sigmoid(att / HW + bias)
            res = small.tile([1, _BPC], f32)
            nc.scalar.activation(
                out=res[:],
                in_=att[:],
                func=mybir.ActivationFunctionType.Sigmoid,
                bias=b_sb[:],
                scale=1.0 / float(_HW),
            )
            nc.sync.dma_start(out=out[:], in_=res[:])

    nc.compile()
    return nc


def _prepare_in_maps(x, W, b):
    xs = np.ascontiguousarray(x, dtype=np.float32).reshape(_B, _C, _HW)
    w_col = np.ascontiguousarray(W, dtype=np.float32).reshape(_C, 1)
    b_col = np.ascontiguousarray(b, dtype=np.float32).reshape(1, 1)
    return [
        {
            "x": np.ascontiguousarray(xs[i * _BPC : (i + 1) * _BPC]),
            "w": w_col,
            "bias": b_col,
        }
        for i in range(_NCORES)
    ]


def _gather(results):
    outs = [np.asarray(results[i]["out"]).reshape(_BPC) for i in range(_NCORES)]
    return np.concatenate(outs, axis=0).reshape(_B, 1, 1, 1).astype(np.float32)


def kernel(x, W, b):
    from concourse.bass_utils import run_bass_kernel_spmd

    global _cached_nc
    if _cached_nc is None:
        _cached_nc = _build_nc()
    in_maps = _prepare_in_maps(x, W, b)
    res = run_bass_kernel_spmd(_cached_nc, in_maps, list(range(_NCORES)))
    return _gather(res.results)


# revision 5
# speedup vs baseline: 1.0225x; 1.0225x over previous
"""ChannelAttentionModule kernel for TRN2 (Bass/Tile), 8-core SPMD.

Computes sigmoid(mean_{hw}(x) @ W.T + b) for x:[16,128,256,256].
Sharding: data-parallel over batch, 2 images per core. Each core streams
its 64 MiB shard once from HBM, reduces over H*W on the vector engine,
contracts channels with W on the tensor engine, applies sigmoid on the
scalar engine. Output [16,1,1,1] gathered on host by concatenation.
"""

import numpy as np

_B, _C, _HW = 16, 128, 65536  # batch, channels, H*W
_NCORES = 8
_BPC = _B // _NCORES  # batches per core = 2
_NCH = 16  # chunks per batch
_F = _HW // _NCH  # free-dim elements per chunk = 4096

_cached_nc = None


def _build_nc():
    import concourse.bacc as bacc
    import concourse.tile as tile
    from concourse import mybir

    f32 = mybir.dt.float32
    nc = bacc.Bacc(
        "TRN2",
        target_bir_lowering=False,
        debug=False,
        num_devices=_NCORES,
    )

    x = nc.dram_tensor("x", [_BPC, _C, _HW], f32, kind="ExternalInput")
    w = nc.dram_tensor("w", [_C, 1], f32, kind="ExternalInput")
    bvec = nc.dram_tensor("bias", [1, 1], f32, kind="ExternalInput")
    out = nc.dram_tensor("out", [1, _BPC], f32, kind="ExternalOutput")

    with tile.TileContext(nc) as tc:
        with (
            tc.tile_pool(name="big", bufs=6) as big,
            tc.tile_pool(name="small", bufs=1) as small,
            tc.tile_pool(name="psum", bufs=1, space="PSUM") as psum,
        ):
            # Tiny loads go via SWDGE (gpsimd) so the HWDGE ring starts
            # streaming x chunks immediately.
            w_sb = small.tile([_C, 1], f32)
            nc.gpsimd.dma_start(out=w_sb[:], in_=w[:])
            b_sb = small.tile([1, 1], f32)
            nc.gpsimd.dma_start(out=b_sb[:], in_=bvec[:])

            partials = small.tile([_C, _BPC * _NCH], f32)
            for bi in range(_BPC):
                for j in range(_NCH):
                    t = big.tile([_C, _F], f32, tag="xtile")
                    nc.sync.dma_start(out=t[:], in_=x[bi, :, j * _F : (j + 1) * _F])
                    k = bi * _NCH + j
                    nc.vector.reduce_sum(
                        out=partials[:, k : k + 1],
                        in_=t[:],
                        axis=mybir.AxisListType.X,
                    )

            # Contract channels: [1, BPC*NCH] = w[C,1].T @ partials[C, BPC*NCH]
            ps = psum.tile([1, _BPC * _NCH], f32)
            nc.tensor.matmul(ps[:], w_sb[:], partials[:])

            # Sum each batch's chunk partials: [1, BPC]
            att = small.tile([1, _BPC], f32)
            nc.vector.reduce_sum(
                out=att[:],
                in_=ps[:].rearrange("p (b j) -> p b j", j=_NCH),
                axis=mybir.AxisListType.X,
            )

            # sigmoid(att / HW + bias)
            res = small.tile([1, _BPC], f32)
            nc.scalar.activation(
                out=res[:],
                in_=att[:],
                func=mybir.ActivationFunctionType.Sigmoid,
                bias=b_sb[:],
                scale=1.0 / float(_HW),
            )
            nc.sync.dma_start(out=out[:], in_=res[:])

    nc.compile()
    return nc


def _prepare_in_maps(x, W, b):
    xs = np.ascontiguousarray(x, dtype=np.float32).reshape(_B, _C, _HW)
    w_col = np.ascontiguousarray(W, dtype=np.float32).reshape(_C, 1)
    b_col = np.ascontiguousarray(b, dtype=np.float32).reshape(1, 1)
    return [
        {
            "x": np.ascontiguousarray(xs[i * _BPC : (i + 1) * _BPC]),
            "w": w_col,
            "bias": b_col,
        }
        for i in range(_NCORES)
    ]


def _gather(results):
    outs = [np.asarray(results[i]["out"]).reshape(_BPC) for i in range(_NCORES)]
    return np.concatenate(outs, axis=0).reshape(_B, 1, 1, 1).astype(np.float32)


def kernel(x, W, b):
    from concourse.bass_utils import run_bass_kernel_spmd

    global _cached_nc
    if _cached_nc is None:
        _cached_nc = _build_nc()
    in_maps = _prepare_in_maps(x, W, b)
    res = run_bass_kernel_spmd(_cached_nc, in_maps, list(range(_NCORES)))
    return _gather(res.results)


# revision 7
# speedup vs baseline: 1.1721x; 1.1463x over previous
"""ChannelAttentionModule kernel for TRN2 (Bass/Tile), 8-core SPMD.

Computes sigmoid(mean_{hw}(x) @ W.T + b) for x:[16,128,256,256].
Sharding: data-parallel over batch, 2 images per core. Each core streams
its 64 MiB shard once from HBM, reduces over H*W on the vector engine,
contracts channels with W on the tensor engine, applies sigmoid on the
scalar engine. Output [16,1,1,1] gathered on host by concatenation.
"""

import numpy as np

_B, _C, _HW = 16, 128, 65536  # batch, channels, H*W
_NCORES = 8
_BPC = _B // _NCORES  # batches per core = 2
_NCH = 16  # chunks per batch
_F = _HW // _NCH  # free-dim elements per chunk = 4096

_cached_nc = None


def _build_nc():
    import concourse.bacc as bacc
    import concourse.tile as tile
    from concourse import mybir

    f32 = mybir.dt.float32
    nc = bacc.Bacc(
        "TRN2",
        target_bir_lowering=False,
        debug=False,
        num_devices=_NCORES,
    )

    # x stored flat per batch; each slab s is a fully contiguous 2 MiB
    # region read as [128, F] with partition p <- slab_start + p*F.
    x = nc.dram_tensor("x", [_BPC, _C * _HW], f32, kind="ExternalInput")
    # Per-slab expanded weights (mean scale folded in on host):
    # wexp[p, s] = W[s*CPS + p//PPC] / HW
    wexp = nc.dram_tensor("wexp", [128, _NCH], f32, kind="ExternalInput")
    bvec = nc.dram_tensor("bias", [1, 1], f32, kind="ExternalInput")
    out = nc.dram_tensor("out", [1, _BPC], f32, kind="ExternalOutput")

    slab = _C * _HW // _NCH  # elements per slab

    with tile.TileContext(nc) as tc:
        with (
            tc.tile_pool(name="big", bufs=6) as big,
            tc.tile_pool(name="small", bufs=1) as small,
            tc.tile_pool(name="psum", bufs=1, space="PSUM") as psum,
        ):
            # Tiny loads go via SWDGE (gpsimd) so the HWDGE ring starts
            # streaming x chunks immediately.
            w_sb = small.tile([128, _NCH], f32)
            nc.gpsimd.dma_start(out=w_sb[:], in_=wexp[:])
            b_sb = small.tile([1, 1], f32)
            nc.gpsimd.dma_start(out=b_sb[:], in_=bvec[:])

            partials = small.tile([128, _BPC, _NCH], f32)
            ps = psum.tile([1, _BPC], f32)
            for s in range(_NCH):
                for bi in range(_BPC):
                    t = big.tile([128, _F], f32, tag="xtile")
                    nc.sync.dma_start(
                        out=t[:],
                        in_=x[bi, s * slab : (s + 1) * slab].rearrange(
                            "(p f) -> p f", f=_F
                        ),
                    )
                    nc.vector.reduce_sum(
                        out=partials[:, bi, s : s + 1],
                        in_=t[:],
                        axis=mybir.AxisListType.X,
                    )
                # Accumulate this slab's weighted partition-contraction
                # into PSUM while the stream continues:
                # ps[0, b] += sum_p wexp[p, s] * partials[p, b, s]
                nc.tensor.matmul(
                    ps[:],
                    w_sb[:, s : s + 1],
                    partials[:, :, s],
                    start=(s == 0),
                    stop=(s == _NCH - 1),
                )

            # sigmoid(att + bias); mean scale already folded into wexp
            res = small.tile([1, _BPC], f32)
            nc.scalar.activation(
                out=res[:],
                in_=ps[:],
                func=mybir.ActivationFunctionType.Sigmoid,
                bias=b_sb[:],
                scale=1.0,
            )
            nc.sync.dma_start(out=out[:], in_=res[:])

    nc.compile()
    return nc


def _prepare_in_maps(x, W, b):
    xs = np.ascontiguousarray(x, dtype=np.float32).reshape(_B, _C * _HW)
    b_col = np.ascontiguousarray(b, dtype=np.float32).reshape(1, 1)
    # wexp[p, s] = W[channel of partition p in slab s] / HW.
    # Slab s covers channels [s*cps, (s+1)*cps); each channel spans
    # ppc = 128 // cps consecutive partitions.
    w_flat = np.asarray(W, dtype=np.float32).reshape(_C)
    cps = _C // _NCH  # channels per slab
    ppc = 128 // cps  # partitions per channel
    ch = (np.arange(_NCH)[None, :] * cps) + (np.arange(128)[:, None] // ppc)
    wexp = np.ascontiguousarray(w_flat[ch] / np.float32(_HW), dtype=np.float32)
    return [
        {
            "x": np.ascontiguousarray(xs[i * _BPC : (i + 1) * _BPC]),
            "wexp": wexp,
            "bias": b_col,
        }
        for i in range(_NCORES)
    ]


def _gather(results):
    outs = [np.asarray(results[i]["out"]).reshape(_BPC) for i in range(_NCORES)]
    return np.concatenate(outs, axis=0).reshape(_B, 1, 1, 1).astype(np.float32)


def kernel(x, W, b):
    from concourse.bass_utils import run_bass_kernel_spmd

    global _cached_nc
    if _cached_nc is None:
        _cached_nc = _build_nc()
    in_maps = _prepare_in_maps(x, W, b)
    res = run_bass_kernel_spmd(_cached_nc, in_maps, list(range(_NCORES)))
    return _gather(res.results)
